# revision 15
# baseline (speedup 1.0000x reference)
"""CapsuleLayer dynamic-routing kernel for 8 Trainium2 NeuronCores.

Problem: x [64,2048,16], route_weights [32,2048,16,32] ->
  3-iteration routing -> out [32,64,1,1,32] (fp32).

Sharding: capsules (C=32) split 4-per-core across 8 cores; x replicated.

Per-core algorithm (J=16, O=32, R=2048, B=64, RJ=32768):
  phase A : s0[c,b,o]  = (1/R) sum_(j,r) x W           (PE, K=128 chunks)
  V pass  : V[c,b,(r,j)] = sum_o W[c,r,j,o] out[c,b,o] (PE, block-diag lhsT,
            2 capsules per MM, K=64 zero-padded to 128)
  delta   : d[c,b,r] = sum_j x*V                       (DVE mult + reduce,
            reduce optionally split to GPSIMD)
  softmax : e = exp(l - max), Z accum                  (DVE max + ACT exp)
  eT      : p2T[r%128, c, rb, b] = e^T                 (PE transpose + ACT)
  xe      : xe[c] = xt2 * eT                           (DVE, fp16)
  s MM    : psS[128,256] += W2^T @ concat_c xe_c       (PE, 1 MM/chunk N=256;
            diagonal 32x64 blocks hold u_c)
  squash  : out_i = squash(u/Z)                        (DVE/ACT)

All HBM streams are fp16 (x, W in both layouts); accumulations f32.
"""
import os
import numpy as np

C, B, R, CIN, OUT = 32, 64, 2048, 16, 32
NCORES = 8
CLOC = C // NCORES          # 4 capsules per core
RJ = R * CIN                # 32768
NCH = RJ // 128             # 256 chunks of 128 along the permuted (j,r) axis
NKB = 8                     # w2cat DMA blocks (4096 rows each, 32 chunks)

_CACHE = {}


def _build_program():
    from contextlib import ExitStack
    import concourse.bass as bass
    import concourse.bacc as bacc
    import concourse.tile as tile
    from concourse import mybir

    f32 = mybir.dt.float32
    f16 = mybir.dt.float16
    AL = mybir.AluOpType
    AF = mybir.ActivationFunctionType
    AX = mybir.AxisListType

    nc = bacc.Bacc(None, target_bir_lowering=False,
                   detect_race_conditions=not bool(int(os.environ.get("CAPS_NO_RACE", "0"))))
    n_loops = int(os.environ.get("CAPS_LOOPS", "1"))
    # fraction control: g % 3 == gps_mod -> reduce on GPSIMD (0..2 on, 3=off)
    gps_on = int(os.environ.get("CAPS_GPS", "1"))

    # ---- DRAM I/O ----
    # w2cat rows permuted: g_host = 4096*kb + 32*p + i, chunk ch = 32*kb + i
    # maps to (j = ch//16, r = 128*(ch%16) + p); columns are (c,o).
    w2cat = nc.dram_tensor("w2cat", [RJ, 128], f16, kind="ExternalInput")
    # xt2[p, ch, b] = x[b, r(ch,p), j(ch)]
    xt2 = nc.dram_tensor("xt2", [128, NCH, B], f16, kind="ExternalInput")
    # wt[c, o, (r,j)] r-major
    wt = nc.dram_tensor("wt", [CLOC, OUT, RJ], f16, kind="ExternalInput")
    # x2d[(2,b), (r,j)] r-major, duplicated along partitions
    x2d = nc.dram_tensor("x2d", [128, RJ], f16, kind="ExternalInput")
    ident = nc.dram_tensor("ident", [128, 128], f32, kind="ExternalInput")
    identh = nc.dram_tensor("identh", [128, 128], f16, kind="ExternalInput")
    out3 = nc.dram_tensor("out3", [B, 128], f32, kind="ExternalOutput")

    with tile.TileContext(nc) as tc, ExitStack() as ctx:
        const = ctx.enter_context(tc.tile_pool(name="const", bufs=1))
        small = ctx.enter_context(tc.tile_pool(name="small", bufs=3))
        w2_p = ctx.enter_context(tc.tile_pool(name="w2p", bufs=2))
        wt_p = ctx.enter_context(tc.tile_pool(name="wtp", bufs=2))
        vs_p = ctx.enter_context(tc.tile_pool(name="vsp", bufs=3))
        vx_p = ctx.enter_context(tc.tile_pool(name="vxp", bufs=4))
        xe_p = ctx.enter_context(tc.tile_pool(name="xep", bufs=3))
        fold_p = ctx.enter_context(tc.tile_pool(name="foldp", bufs=2))
        psV_p = ctx.enter_context(tc.tile_pool(name="psV", bufs=4, space="PSUM"))
        psS_p = ctx.enter_context(tc.tile_pool(name="psS", bufs=1, space="PSUM"))
        psT_p = ctx.enter_context(tc.tile_pool(name="psT", bufs=1, space="PSUM"))
        psTh_p = ctx.enter_context(tc.tile_pool(name="psTh", bufs=2, space="PSUM"))

        # resident x in both layouts; xt2 quarters interleave with the phase A
        # w2cat stream; x2d quarters go on the gpsimd queue (needed ~40us in)
        xt2_sb = const.tile([128, NCH, B], f16, tag="xt2sb", name="xt2_sb")
        idn = const.tile([128, 128], f32, tag="ident", name="idn")
        nc.scalar.dma_start(out=idn, in_=ident[:])
        idnh = const.tile([128, 128], f16, tag="identh", name="idnh")
        nc.scalar.dma_start(out=idnh, in_=identh[:])
        x2d_sb = const.tile([128, RJ], f16, tag="x2dsb", name="x2d_sb")

        # logits per capsule-pair [(2c,b)=128, r=2048]
        lP = [const.tile([128, R], f32, tag=f"l{p}", name=f"lP{p}") for p in range(2)]
        # transposed probs factor e^T [128=r%128, (c=4, rb=16, b=64)] fp16
        p2T = const.tile([128, CLOC, R // 128, B], f16, tag="p2T", name="p2T")
        # block-diag stationaries for the V matmuls (2 capsules each, K=64)
        bd = [const.tile([64, 128], f16, tag=f"bd{p}", name=f"bd{p}") for p in range(2)]
        for p in range(2):
            nc.vector.memset(bd[p], 0.0)
        rzq4 = const.tile([B, CLOC], f32, tag="rzq4", name="rzq4")

        def squash(u_bT, scale_pow):
            """u_bT [64,(4c,32o)] f32: s = u*scale_pow; out = s*sqrt(n2)/(n2+1).

            Returns (out_i [B,128] f32, oT [128,(b)] f16 = out_i^T) and fills
            the block-diag stationaries bd[0], bd[1].
            """
            sq = small.tile([B, 128], f32, tag="sq", name="sq")
            nc.vector.scalar_tensor_tensor(
                out=sq, in0=u_bT, scalar=float(scale_pow * scale_pow),
                in1=u_bT, op0=AL.mult, op1=AL.mult)
            n2 = small.tile([B, CLOC], f32, tag="n2", name="n2")
            nc.vector.tensor_reduce(
                out=n2, in_=sq[:].rearrange("b (c o) -> b c o", c=CLOC),
                axis=AX.X, op=AL.add)
            rt = small.tile([B, CLOC], f32, tag="rt", name="rt")
            nc.scalar.activation(out=rt, in_=n2, func=AF.Sqrt)
            dn = small.tile([B, CLOC], f32, tag="dn", name="dn")
            nc.vector.tensor_scalar_add(out=dn, in0=n2, scalar1=1.0)
            rc = small.tile([B, CLOC], f32, tag="rc", name="rc")
            nc.vector.reciprocal(out=rc, in_=dn)
            f = small.tile([B, CLOC], f32, tag="f", name="f")
            nc.vector.tensor_mul(out=f, in0=rt, in1=rc)
            f2 = small.tile([B, CLOC], f32, tag="f2", name="f2")
            nc.vector.tensor_scalar_mul(out=f2, in0=f, scalar1=float(scale_pow))
            o_i = small.tile([B, 128], f32, tag="oi", name="oi")
            f2b = bass.AP(tensor=f2[:].tensor, offset=f2[:].offset,
                          ap=[f2[:].ap[0], f2[:].ap[1], [0, OUT]])
            nc.vector.tensor_tensor(
                out=o_i[:].rearrange("b (c o) -> b c o", c=CLOC),
                in0=u_bT[:].rearrange("b (c o) -> b c o", c=CLOC),
                in1=f2b, op=AL.mult)
            psOT = psT_p.tile([128, B], f32, tag="psT", name="psOT")
            nc.tensor.transpose(psOT, o_i, idn[0:B, 0:B])
            oT = small.tile([128, B], f16, tag="oT", name="oT")
            nc.scalar.copy(out=oT, in_=psOT)
            # block-diag stationaries (pair p): rows 0-31 = (c=2p, o) block for
            # cols b of c=2p; rows 32-63 = (c=2p+1, o) block for cols of 2p+1
            for p in range(2):
                nc.scalar.copy(out=bd[p][0:32, 0:64],
                               in_=oT[64 * p:64 * p + 32, :])
                nc.scalar.copy(out=bd[p][32:64, 64:128],
                               in_=oT[64 * p + 32:64 * p + 64, :])
            return o_i, oT

        for _loop in range(n_loops):
            # ---------- Phase A: s0 = (1/R) sum_(j,r) x W ----------
            psA = psS_p.tile([128, 256], f32, tag="psS", name="psA")
            for kb in range(16):
                if _loop == 0 and kb % 4 == 0:
                    q = kb // 4
                    nc.scalar.dma_start(
                        out=xt2_sb[:, 64 * q:64 * (q + 1), :],
                        in_=xt2[:, 64 * q:64 * (q + 1), :])
                if _loop == 0 and kb in (2, 6, 10, 14):
                    q = kb // 4
                    nc.gpsimd.dma_start(
                        out=x2d_sb[0:64, 8192 * q:8192 * (q + 1)],
                        in_=x2d[0:64, 8192 * q:8192 * (q + 1)])
                    nc.gpsimd.dma_start(
                        out=x2d_sb[64:128, 8192 * q:8192 * (q + 1)],
                        in_=x2d_sb[0:64, 8192 * q:8192 * (q + 1)])
                w2k = w2_p.tile([128, 16, 128], f16, tag="w2k", name="w2k")
                nc.sync.dma_start(
                    out=w2k,
                    in_=w2cat[2048 * kb:2048 * (kb + 1), :].rearrange(
                        "(p i) n -> p i n", p=128))
                for i in range(16):
                    ch = 16 * kb + i
                    nc.tensor.matmul(psA[:, 0:B], w2k[:, i, :], xt2_sb[:, ch, :],
                                     start=(ch == 0), stop=(ch == NCH - 1))
            sA = small.tile([128, B], f32, tag="sA", name="sA")
            nc.scalar.copy(out=sA, in_=psA[:, 0:B])
            psAT = psT_p.tile([B, 128], f32, tag="psT", name="psAT")
            nc.tensor.transpose(psAT, sA, idn)
            uT = small.tile([B, 128], f32, tag="uT", name="uT")
            nc.scalar.copy(out=uT, in_=psAT)
            out_i, oT = squash(uT, 1.0 / R)

            # ---------- Two routing boundaries (pair-de-interleaved) ----------
            for it in (1, 2):
                zq = small.tile([B, CLOC], f32, tag="zq", name="zq")
                for pr in range(2):
                    # --- V matmuls + delta for this pair only (K=64) ---
                    for g in range(16):
                        wtk = wt_p.tile([64, 2048], f16, tag="wtk", name="wtk")
                        nc.sync.dma_start(
                            out=wtk,
                            in_=wt[2 * pr:2 * pr + 2, :, 2048 * g:2048 * (g + 1)]
                            .rearrange("c o n -> (c o) n"))
                        vs = vs_p.tile([128, 2048], f16, tag="vs", name="vs")
                        for t in range(4):
                            psV0 = psV_p.tile([128, 512], f32, tag="psV",
                                              name="psV0")
                            nc.tensor.matmul(psV0, bd[pr],
                                             wtk[:, 512 * t:512 * (t + 1)],
                                             start=True, stop=True)
                            nc.scalar.copy(out=vs[:, 512 * t:512 * (t + 1)],
                                           in_=psV0)
                        vx = vx_p.tile([128, 2048], f16, tag="vx", name="vx")
                        nc.vector.tensor_mul(
                            out=vx, in0=vs,
                            in1=x2d_sb[:, 2048 * g:2048 * (g + 1)])
                        v3 = vx[:].rearrange("p (r j) -> p r j", j=CIN)
                        lslc = lP[pr][:, 128 * g:128 * (g + 1)].rearrange(
                            "p (r u) -> p r u", u=1)
                        f8 = fold_p.tile([128, 128, 8], f16, tag="f8", name="f8")
                        nc.vector.tensor_add(out=f8, in0=v3[:, :, 0:8],
                                             in1=v3[:, :, 8:16])
                        f4 = fold_p.tile([128, 128, 4], f16, tag="f4", name="f4")
                        nc.vector.tensor_add(out=f4, in0=f8[:, :, 0:4],
                                             in1=f8[:, :, 4:8])
                        f2 = fold_p.tile([128, 128, 2], f16, tag="f2t",
                                         name="f2t")
                        nc.gpsimd.tensor_add(out=f2, in0=f4[:, :, 0:2],
                                             in1=f4[:, :, 2:4])
                        if it == 1:
                            nc.gpsimd.tensor_add(out=lslc, in0=f2[:, :, 0:1],
                                                 in1=f2[:, :, 1:2])
                        else:
                            dtmp = fold_p.tile([128, 128, 1], f32, tag="dtmp3",
                                               name="dtmp3")
                            nc.gpsimd.tensor_add(out=dtmp, in0=f2[:, :, 0:1],
                                                 in1=f2[:, :, 1:2])
                            nc.vector.tensor_add(out=lslc, in0=lslc, in1=dtmp)

                    # --- softmax pieces for this pair ---
                    m = small.tile([128, 1], f32, tag="m", name="m")
                    nc.vector.tensor_reduce(out=m, in_=lP[pr], axis=AX.X,
                                            op=AL.max)
                    mneg = small.tile([128, 1], f32, tag="mneg", name="mneg")
                    nc.vector.tensor_scalar_mul(out=mneg, in0=m, scalar1=-1.0)
                    eP = vx_p.tile([128, R], f16, tag="vx", name="eP")
                    Z = small.tile([128, 1], f32, tag="Z", name="Z")
                    nc.scalar.activation(out=eP, in_=lP[pr], func=AF.Exp,
                                         bias=mneg[:, 0:1], scale=1.0,
                                         accum_out=Z)
                    for ce in range(2):
                        nc.gpsimd.dma_start(
                            out=zq[:, 2 * pr + ce:2 * pr + ce + 1],
                            in_=Z[64 * ce:64 * (ce + 1), 0:1])
                    for rb in range(R // 128):
                        psT2 = psTh_p.tile([128, 128], f16, tag="psTh",
                                           name="psT2")
                        nc.tensor.transpose(
                            psT2, eP[:, 128 * rb:128 * (rb + 1)], idnh)
                        nc.scalar.copy(
                            out=p2T[:, 2 * pr:2 * pr + 2, rb, :],
                            in_=psT2[:].rearrange("p (c b) -> p c b", c=2))
                nc.vector.reciprocal(out=rzq4, in_=zq)

                # --- xe + s matmuls: two 256-chunk chains (pair A, then B) ---
                psS = psS_p.tile([128, 256], f32, tag="psS", name="psS")
                for pair in range(2):
                    w2k = None
                    for j in range(CIN):
                        xe2 = xe_p.tile([128, R // 128, 2, B], f16,
                                        tag=f"xe{pair}", name=f"xe{pair}")
                        for ce in range(2):
                            xeng = (nc.gpsimd
                                    if (gps_on and ce == 1 and pair == 0
                                        and j % 2 == 0) else nc.vector)
                            xeng.tensor_mul(
                                out=xe2[:, :, ce, :],
                                in0=xt2_sb[:, 16 * j:16 * (j + 1), :],
                                in1=p2T[:, 2 * pair + ce, :, :])
                        for rb in range(R // 128):
                            ch = 16 * j + rb
                            if rb == 0:
                                w2k = w2_p.tile([128, 16, 128], f16, tag="w2k",
                                                name="w2k")
                                nc.sync.dma_start(
                                    out=w2k,
                                    in_=w2cat[2048 * j:2048 * (j + 1), :]
                                    .rearrange("(p i) n -> p i n", p=128))
                            nc.tensor.matmul(
                                psS[:, 128 * pair:128 * (pair + 1)],
                                w2k[:, rb, :],
                                xe2[:, rb, :, :].rearrange("p c b -> p (c b)"),
                                start=(ch == 0), stop=(ch == NCH - 1))
                # diagonal blocks -> sS4 [(c,o), b] f32, transpose, scale by 1/Z
                sS4 = small.tile([128, B], f32, tag="sS4", name="sS4")
                for c4 in range(CLOC):
                    nc.scalar.copy(
                        out=sS4[32 * c4:32 * (c4 + 1), :],
                        in_=psS[32 * c4:32 * (c4 + 1), 64 * c4:64 * (c4 + 1)])
                psU = psT_p.tile([B, 128], f32, tag="psT", name="psU")
                nc.tensor.transpose(psU, sS4, idn)
                uT2 = small.tile([B, 128], f32, tag="uT", name="uT2")
                for c4 in range(CLOC):
                    nc.scalar.activation(
                        out=uT2[:, 32 * c4:32 * (c4 + 1)],
                        in_=psU[:, 32 * c4:32 * (c4 + 1)],
                        func=AF.Copy, bias=0.0, scale=rzq4[:, c4:c4 + 1])
                out_i, oT = squash(uT2, 1.0)

            nc.sync.dma_start(out=out3[:], in_=out_i)

    nc.finalize()
    return nc


def _get_program():
    if "nc" not in _CACHE:
        _CACHE["nc"] = _build_program()
    return _CACHE["nc"]


def make_in_maps(x, route_weights):
    x = np.ascontiguousarray(x, dtype=np.float32)
    W = np.ascontiguousarray(route_weights, dtype=np.float32)

    # xt2[p, ch, b] with ch = 16*j + rb, r = 128*rb + p
    xt = x.transpose(2, 1, 0).reshape(CIN, R // 128, 128, B)   # [j, rb, p, b]
    xt2 = np.ascontiguousarray(
        xt.transpose(2, 0, 1, 3).reshape(128, NCH, B)).astype(np.float16)

    xnat = x.reshape(B, RJ)                                    # [b,(r,j)]
    x2d = np.ascontiguousarray(
        np.concatenate([xnat, xnat], axis=0)).astype(np.float16)
    ident = np.eye(128, dtype=np.float32)
    identh = np.eye(128, dtype=np.float16)

    in_maps = []
    for core in range(NCORES):
        wc = W[CLOC * core:CLOC * (core + 1)]                  # [4,R,J,O]
        # w2cat rows: g = 2048*j + 16*p + rb  (ch = 16*j + rb, r = 128*rb + p)
        t = wc.reshape(CLOC, R // 128, 128, CIN, OUT)          # [c, rb, p, j, o]
        t = t.transpose(3, 2, 1, 0, 4)                         # [j, p, rb, c, o]
        w2cat = np.ascontiguousarray(
            t.reshape(RJ, CLOC * OUT)).astype(np.float16)
        wtc = np.ascontiguousarray(
            wc.transpose(0, 3, 1, 2).reshape(CLOC, OUT, RJ)).astype(np.float16)
        m = {"w2cat": w2cat, "xt2": xt2, "wt": wtc, "x2d": x2d,
             "ident": ident, "identh": identh}
        in_maps.append(m)
    return in_maps


def kernel(x, route_weights):
    from concourse.bass_utils import run_bass_kernel_spmd

    in_maps = make_in_maps(x, route_weights)
    nc = _get_program()
    res = run_bass_kernel_spmd(nc, in_maps, core_ids=list(range(NCORES)))
    if os.environ.get("CAPS_RESULT_STASH"):
        _CACHE["last_result"] = res

    out = np.empty((C, B, 1, 1, OUT), dtype=np.float32)
    for core in range(NCORES):
        o = res.results[core]["out3"].reshape(B, CLOC, OUT).transpose(1, 0, 2)
        out[CLOC * core:CLOC * (core + 1), :, 0, 0, :] = o
    return out
